# revision 1
# baseline (speedup 1.0000x reference)
"""Contrastive-loss kernel v3: host pre-gathered pairs, PE reductions.

Reference semantics (B=4, N=4096, D=128, T=0.1):
    u = emb / max(||emb||, 1e-12)
    pos_sim[b,n] = dot(u[b,n], u[b, pos_idx[b,n]]) / T
    loss = mean(softplus(-pos_sim)) + mean(softplus(neg_sim))

Sharding: each of 8 cores takes half the rows ("slots") of one batch
element. The partner rows for each slot are known from the index
inputs, so the host shards them directly to each core (an index-select;
all arithmetic stays on device): per slot the core receives the own
row, the positive partner row and the negative partner row, all in
transposed [D=128 partitions, slot] layout, interleaved in slot-chunks
so the kernel streams one linear DMA per chunk at full DMA efficiency
(no 256B-descriptor gather penalty, no on-device descriptor
generation).

Per chunk the vector engines form bf16 products/squares (DVE 2x mode,
squares split over ACT and Pool), and every per-slot reduction over D
is a 1-column PE matmul with the 128-slot block as stationary weights:

    matmul(psum[:, col], lhsT=blk, rhs=(+-1/T | 1))

which also folds the 1/T scale and pos-side negation. The z-chain then
runs on [128, cols] distributed tiles:

    z   = (+-dot/T) * exp(-.5 ln ssq_own) * exp(-.5 ln ssq_partner)
    acc = sum ln(exp(z) + 1)          # softplus, +1 via Ln bias

with a single pinned activation table (Square/Ln/Exp). Output is a
[128, n_groups] per-core partial-sum tile; the host sums / (B*N).
"""

import numpy as np

B, N, D = 4, 4096, 128
NCORES = 8
HALF = N // 2            # slots per core
TEMP = 0.1
CHUNKS = (384, 512, 512, 512, 128)     # slots per chunk (each % 128 == 0)
ZGROUP = (2, 3)                        # chunks per z-group

_PROG = None


def _pin_act_table(table_name="natural_log_exp_and_others"):
    """Make Square/Ln/Exp resolve only to `table_name` so a single act
    table load serves the whole kernel."""
    import functools
    import concourse.hw_specs as hw_specs
    import concourse.bacc as bacc
    import concourse.mybir as mybir

    if getattr(_pin_act_table, "_done", False):
        return
    orig = hw_specs.get_activation_tables
    AF = mybir.ActivationFunctionType
    pinned = {AF.Square, AF.Ln, AF.Exp}

    @functools.cache
    def patched(arch):
        return {k: (v if k == table_name else v - pinned)
                for k, v in orig(arch).items()}

    hw_specs.get_activation_tables = patched
    bacc.get_activation_tables = patched
    _pin_act_table._done = True


def _build_program():
    import concourse.bacc as bacc
    import concourse.tile as tile
    import concourse.mybir as mybir

    _pin_act_table()

    f32 = mybir.dt.float32
    bf16 = mybir.dt.bfloat16
    mult = mybir.AluOpType.mult
    AF = mybir.ActivationFunctionType

    assert sum(CHUNKS) == HALF and sum(ZGROUP) == len(CHUNKS)
    NG = len(ZGROUP)
    c_off = [sum(CHUNKS[:i]) for i in range(len(CHUNKS))]
    # z-group of each chunk + column offset (in 128-blocks) within group
    g_of, g_boff, g_nb = [], [], []
    k = 0
    for g, n in enumerate(ZGROUP):
        nb = sum(CHUNKS[k + i] for i in range(n)) // 128
        g_nb.append(nb)
        off = 0
        for i in range(n):
            g_of.append(g)
            g_boff.append(off)
            off += CHUNKS[k + i] // 128
        k += n

    nc = bacc.Bacc("TRN2", target_bir_lowering=False)

    data = nc.dram_tensor("data", [128, 3 * HALF], bf16, kind="ExternalInput")
    out = nc.dram_tensor("partial", [128, NG], f32, kind="ExternalOutput")

    with tile.TileContext(nc) as tc:
        with tc.tile_pool(name="sb", bufs=1) as pool, \
             tc.tile_pool(name="ps", bufs=1, space="PSUM") as psum:

            ones = pool.tile([128, 3], bf16, tag="ones")   # [+1, -1/T, +1/T]
            nc.vector.memset(ones[:, 0:1], 1.0)
            nc.vector.memset(ones[:, 1:2], -1.0 / TEMP)
            nc.vector.memset(ones[:, 2:3], 1.0 / TEMP)

            chunks = []
            for k, C in enumerate(CHUNKS):
                t = pool.tile([128, 3 * C], bf16, tag=f"d{k}")
                nc.sync.dma_start(out=t[:], in_=data[:, 3 * c_off[k]:3 * (c_off[k] + C)])
                chunks.append(t)

            # group psum layout (nb = g_nb[g] 128-blocks):
            #   [0:nb]      ssq pos-partner      [3nb:4nb]  ssq own (dup)
            #   [nb:2nb]    ssq neg-partner      [4nb:5nb]  dots pos (* -1/T)
            #   [2nb:3nb]   ssq own              [5nb:6nb]  dots neg (* +1/T)
            q_t = []
            for g in range(NG):
                qg = psum.tile([128, 6 * g_nb[g]], f32, tag=f"q{g}")
                q_t.append(qg)
            acc = pool.tile([128, NG], f32, tag="acc")

            def emit_B(k, C):
                g, nb, nbg = g_of[k], C // 128, g_nb[g_of[k]]
                bo = g_boff[k]
                q = q_t[g]
                o_v = chunks[k][:, 0:C]
                gp_v = chunks[k][:, C:2 * C]
                gn_v = chunks[k][:, 2 * C:3 * C]
                sqo = pool.tile([128, C], bf16, tag=f"sqo{k}")
                nc.scalar.square(sqo[:], o_v)
                sgp = pool.tile([128, C], bf16, tag=f"sgp{k}")
                if k == len(CHUNKS) - 1:
                    nc.vector.tensor_tensor(out=sgp[:], in0=gp_v, in1=gp_v, op=mult)
                else:
                    nc.gpsimd.tensor_tensor(out=sgp[:], in0=gp_v, in1=gp_v, op=mult)
                sgn = pool.tile([128, C], bf16, tag=f"sgn{k}")
                nc.vector.tensor_tensor(out=sgn[:], in0=gn_v, in1=gn_v, op=mult)
                pp = pool.tile([128, C], bf16, tag=f"pp{k}")
                nc.vector.tensor_tensor(out=pp[:], in0=o_v, in1=gp_v, op=mult)
                pn = pool.tile([128, C], bf16, tag=f"pn{k}")
                nc.vector.tensor_tensor(out=pn[:], in0=o_v, in1=gn_v, op=mult)
                for j in range(nb):
                    bl = slice(j * 128, (j + 1) * 128)
                    col = bo + j
                    for base, t_, rhs in ((0, sgp, 0), (nbg, sgn, 0),
                                          (2 * nbg, sqo, 0), (3 * nbg, sqo, 0),
                                          (4 * nbg, pp, 1), (5 * nbg, pn, 2)):
                        nc.tensor.matmul(q[:, base + col:base + col + 1],
                                         t_[:, bl], ones[:, rhs:rhs + 1],
                                         start=True, stop=True)

            def emit_Z(g):
                nbg = g_nb[g]
                q = q_t[g]
                lnq = pool.tile([128, 4 * nbg], f32, tag=f"ln{g}")
                nc.scalar.activation(lnq[:], q[:, 0:4 * nbg], AF.Ln)
                rinv = pool.tile([128, 4 * nbg], f32, tag=f"ri{g}")
                nc.scalar.activation(rinv[:], lnq[:], AF.Exp, scale=-0.5)
                ct = pool.tile([128, 2 * nbg], f32, tag=f"ct{g}")
                nc.vector.tensor_tensor(
                    out=ct[:], in0=q[:, 4 * nbg:6 * nbg], in1=rinv[:, 0:2 * nbg], op=mult)
                z2 = pool.tile([128, 2 * nbg], f32, tag=f"z2{g}")
                nc.vector.tensor_tensor(
                    out=z2[:], in0=ct[:], in1=rinv[:, 2 * nbg:4 * nbg], op=mult)
                ez = pool.tile([128, 2 * nbg], f32, tag=f"ez{g}")
                nc.scalar.activation(ez[:], z2[:], AF.Exp)
                sp = pool.tile([128, 2 * nbg], f32, tag=f"sp{g}")
                nc.scalar.activation(sp[:], ez[:], AF.Ln, bias=1.0,
                                     accum_out=acc[:, g:g + 1])

            last_of_group = {}
            for k in range(len(CHUNKS)):
                last_of_group[g_of[k]] = k
            for k, C in enumerate(CHUNKS):
                emit_B(k, C)
                for g in range(NG - 1):
                    if last_of_group[g] == k - 1:
                        emit_Z(g)
            emit_Z(NG - 1)

            nc.sync.dma_start(out=out[:], in_=acc[:])

    nc.compile()
    return nc


def _get_program():
    global _PROG
    if _PROG is None:
        _PROG = _build_program()
    return _PROG


def _shard_inputs(embeddings, positive_pairs, negative_pairs):
    import ml_dtypes

    emb = np.asarray(embeddings, dtype=np.float32)
    emb_bf = emb.astype(ml_dtypes.bfloat16)
    pos = np.asarray(positive_pairs).reshape(B, N)
    neg = np.asarray(negative_pairs).reshape(B, N)

    in_maps = []
    for c in range(NCORES):
        b, h = divmod(c, 2)
        base = h * HALF
        E = emb_bf[b]
        o_t = E[base:base + HALF].T          # [128, HALF]
        gp_t = E[pos[b, base:base + HALF]].T
        gn_t = E[neg[b, base:base + HALF]].T
        blocks = []
        for k, C in enumerate(CHUNKS):
            s = slice(sum(CHUNKS[:k]), sum(CHUNKS[:k]) + C)
            blocks += [o_t[:, s], gp_t[:, s], gn_t[:, s]]
        in_maps.append({"data": np.ascontiguousarray(np.concatenate(blocks, axis=1))})
    return in_maps


def kernel(embeddings, positive_pairs, negative_pairs):
    from concourse.bass_utils import run_bass_kernel_spmd

    nc = _get_program()
    in_maps = _shard_inputs(embeddings, positive_pairs, negative_pairs)
    res = run_bass_kernel_spmd(nc, in_maps, core_ids=list(range(NCORES)))
    total = sum(r["partial"].astype(np.float64).sum() for r in res.results)
    return np.float32(total / (B * N))

